# revision 3
# baseline (speedup 1.0000x reference)
"""AtomicComposition histogram kernel for 8 TRN2 NeuronCores.

Reference semantics (nn_AtomicComposition): for each structure (contiguous
256-atom block), count atoms whose atomic number is in ALL_SPECIES =
[1, 6, 7, 8, 16] -> output (32768, 5) float32.

Sharding: data-parallel over structures. Each of the 8 cores gets a
contiguous block of 4096 structures (1048576 atoms). No cross-core
reduction (structures are disjoint).

Device algorithm per core:
  - species shard viewed as [4 tiles, 128 partitions, 2048 atoms]
    (partition row = 8 consecutive structures of 256 atoms each)
  - gpsimd casting DMA loads int32 DRAM -> bfloat16 SBUF (values < 119 are
    exact in bf16)
  - for each 256-atom segment and each of the 5 species values, one fused
    DVE tensor_scalar(is_equal, accum_out) computes the count directly
    (runs in 4x DVE perf mode: bf16, SBUF, single-src)
  - counts tile [128, 40] f32 DMAed to the output
"""

import numpy as np

import concourse.bass as bass
import concourse.mybir as mybir
from concourse.bacc import Bacc
from concourse.tile import TileContext
from concourse.bass_utils import run_bass_kernel_spmd

N_CORES = 8
N_STRUCTURES = 32768
ATOMS_PER = 256
S_LOCAL = N_STRUCTURES // N_CORES          # 4096 structures per core
ATOMS_LOCAL = S_LOCAL * ATOMS_PER          # 1048576 atoms per core
ALL_SPECIES = (1, 6, 7, 8, 16)
N_SPECIES = len(ALL_SPECIES)

P = 128                                    # SBUF partitions
TILE_FREE = 2048                           # atoms per partition row per tile
STRUCTS_PER_ROW = TILE_FREE // ATOMS_PER   # 8
N_TILES = ATOMS_LOCAL // (P * TILE_FREE)   # 4


def build_graph(species_vals=ALL_SPECIES):
    nc = Bacc()

    species = nc.declare_dram_parameter(
        "species", [ATOMS_LOCAL], mybir.dt.int32, isOutput=False
    )
    out = nc.declare_dram_parameter(
        "out", [S_LOCAL, N_SPECIES], mybir.dt.float32, isOutput=True
    )

    # [4, 128, 2048] view of the flat species shard
    sp_view = species[:].rearrange("(t p f) -> t p f", p=P, f=TILE_FREE)
    # [4, 128, 40] view of the output: struct s = t*1024 + p*8 + j,
    # column = j*5 + k
    out_view = out[:].rearrange(
        "(t p j) k -> t p (j k)", t=N_TILES, p=P, j=STRUCTS_PER_ROW
    )

    with TileContext(nc) as tc:
        with (
            tc.tile_pool(name="sp", bufs=3) as sp_pool,
            tc.tile_pool(name="cnt", bufs=3) as cnt_pool,
            tc.tile_pool(name="junk", bufs=2) as junk_pool,
        ):
            for t in range(N_TILES):
                tile = sp_pool.tile([P, TILE_FREE], mybir.dt.bfloat16)
                # casting DMA (SWDGE): int32 DRAM -> bf16 SBUF
                nc.gpsimd.dma_start(out=tile[:], in_=sp_view[t])

                cnt = cnt_pool.tile([P, STRUCTS_PER_ROW * N_SPECIES],
                                    mybir.dt.float32)
                junk = junk_pool.tile([P, ATOMS_PER], mybir.dt.bfloat16)

                for j in range(STRUCTS_PER_ROW):
                    seg = tile[:, j * ATOMS_PER:(j + 1) * ATOMS_PER]
                    for k, z in enumerate(species_vals):
                        col = j * N_SPECIES + k
                        nc.vector.tensor_scalar(
                            out=junk[:],
                            in0=seg,
                            scalar1=float(z),
                            scalar2=0.0,
                            op0=mybir.AluOpType.is_equal,
                            op1=mybir.AluOpType.add,
                            accum_out=cnt[:, col:col + 1],
                        )

                nc.sync.dma_start(out=out_view[t], in_=cnt[:])

    nc.finalize()
    return nc


_GRAPH_CACHE = {}


def _get_graph(species_vals):
    key = tuple(int(v) for v in species_vals)
    if key not in _GRAPH_CACHE:
        _GRAPH_CACHE[key] = build_graph(key)
    return _GRAPH_CACHE[key]


def kernel(**inputs) -> np.ndarray:
    species = np.ascontiguousarray(np.asarray(inputs["species"], dtype=np.int32))
    all_species = np.asarray(inputs["all_species"]).reshape(-1)
    assert species.shape == (N_STRUCTURES * ATOMS_PER,), species.shape

    nc = _get_graph(all_species)

    shards = species.reshape(N_CORES, ATOMS_LOCAL)
    in_maps = [{"species": shards[i]} for i in range(N_CORES)]
    res = run_bass_kernel_spmd(nc, in_maps, core_ids=list(range(N_CORES)))
    outs = [np.asarray(res.results[i]["out"]) for i in range(N_CORES)]
    return np.concatenate(outs, axis=0).astype(np.float32)


# revision 8
# speedup vs baseline: 2.1058x; 2.1058x over previous
"""AtomicComposition histogram kernel for 8 TRN2 NeuronCores.

Reference semantics (nn_AtomicComposition): for each structure (contiguous
256-atom block), count atoms whose atomic number is in ALL_SPECIES =
[1, 6, 7, 8, 16] -> output (32768, 5) float32.

Sharding: data-parallel over structures; each core gets 4096 contiguous
structures (1048576 atoms). The host hands each core its shard TRANSPOSED
([256 atom-slots, 4096 structures], int32) so that on-device the segmented
reduction runs on the TensorEngine:

  - gpsimd casting DMA: int32 DRAM -> bf16 SBUF tiles [128, 1024]
    (two partition groups: atom slots 0-127 / 128-255)
  - VectorE: 5 is_equal compares per tile into a 5-plane mask tile
    [128, 5*1024] (bf16, 4x DVE perf mode)
  - TensorE: ones[128,1]^T @ mask_chunk[128, 512] -> PSUM [1, 512]
    accumulated over the two atom-slot groups; chunks parked at
    32-aligned PSUM partitions
  - ScalarE evacuates PSUM -> SBUF; DMA writes the per-core output
    in species-major layout [5, 4096] f32

The host reassembles/transposes to (32768, 5).
"""

import numpy as np

import concourse.bass as bass
import concourse.mybir as mybir
from concourse.bacc import Bacc
from concourse.tile import TileContext
from concourse.bass_utils import run_bass_kernel_spmd

N_CORES = 8
N_STRUCTURES = 32768
ATOMS_PER = 256
S_LOCAL = N_STRUCTURES // N_CORES          # 4096 structures per core
ATOMS_LOCAL = S_LOCAL * ATOMS_PER          # 1048576 atoms per core
ALL_SPECIES = (1, 6, 7, 8, 16)
N_SPECIES = len(ALL_SPECIES)

P = 128
SBLK = 1024                                # structure columns per block
N_BLK = S_LOCAL // SBLK                    # 4
N_GROUPS = ATOMS_PER // P                  # 2 atom-slot groups


def build_graph(species_vals=ALL_SPECIES):
    nsp = len(species_vals)
    nc = Bacc()

    species = nc.declare_dram_parameter(
        "species_t", [ATOMS_PER, S_LOCAL], mybir.dt.int32, isOutput=False
    )
    # species-major output; host transposes back
    out = nc.declare_dram_parameter(
        "out_t", [nsp, S_LOCAL], mybir.dt.float32, isOutput=True
    )

    with TileContext(nc) as tc:
        with (
            tc.tile_pool(name="const", bufs=1) as const_pool,
            tc.tile_pool(name="sp", bufs=3) as sp_pool,
            tc.tile_pool(name="mask", bufs=4) as mask_pool,
            tc.tile_pool(name="psum", bufs=2, space="PSUM") as psum_pool,
            tc.tile_pool(name="evac", bufs=2) as evac_pool,
        ):
            ones = const_pool.tile([P, 1], mybir.dt.bfloat16)
            nc.vector.memset(ones[:], 1.0)

            for c in range(N_BLK):
                masks = []
                for g in range(N_GROUPS):
                    tile = sp_pool.tile([P, SBLK], mybir.dt.bfloat16)
                    nc.gpsimd.dma_start(
                        out=tile[:],
                        in_=species[g * P:(g + 1) * P, c * SBLK:(c + 1) * SBLK],
                    )
                    mask5 = mask_pool.tile([P, nsp * SBLK], mybir.dt.bfloat16)
                    for k, z in enumerate(species_vals):
                        nc.vector.tensor_scalar(
                            out=mask5[:, k * SBLK:(k + 1) * SBLK],
                            in0=tile[:],
                            scalar1=float(z),
                            scalar2=None,
                            op0=mybir.AluOpType.is_equal,
                        )
                    masks.append(mask5)

                # 10 chunks of 512 columns; chunk m = (species m//2, half m%2).
                # Chunks 0-7 -> psum tile a at (partition 32*(m//2), col 512*(m%2));
                # chunks 8-9 -> psum tile b at (partition 0, col 512*(m%2)).
                ps_a = psum_pool.tile([P, 2 * 512], mybir.dt.float32, tag="ps_a")
                ps_b = psum_pool.tile([P, 2 * 512], mybir.dt.float32, tag="ps_b")
                n_chunks = 2 * nsp
                for m in range(n_chunks):
                    z, h = divmod(m, 2)
                    if z < 4:
                        dst = ps_a[32 * z:32 * z + 1, 512 * h:512 * (h + 1)]
                        tpos = (0, 32 * z)
                    else:
                        dst = ps_b[0:1, 512 * h:512 * (h + 1)]
                        tpos = (0, 0)
                    for g in range(N_GROUPS):
                        nc.tensor.matmul(
                            out=dst,
                            lhsT=ones[:],
                            rhs=masks[g][:, 512 * m:512 * (m + 1)],
                            start=(g == 0),
                            stop=(g == N_GROUPS - 1),
                            tile_position=tpos,
                        )

                # evacuate full psum tiles -> sbuf (ScalarE; cost is
                # free-dim-based, unused partitions are free), then DMA
                # only the meaningful rows (DMA may stride partitions)
                ev_a = evac_pool.tile([P, 2 * 512], mybir.dt.float32, tag="ev_a")
                ev_b = evac_pool.tile([P, 2 * 512], mybir.dt.float32, tag="ev_b")
                nc.scalar.copy(out=ev_a[:], in_=ps_a[:])
                nc.scalar.copy(out=ev_b[0:1, :], in_=ps_b[0:1, :])

                # rows z=0..3 of ev_a (at partitions 32z) each hold 1024
                # counts for structs [c*1024, (c+1)*1024); row 0 of ev_b
                # holds species 4.
                ea = ev_a[:].rearrange("(zz r) q -> zz r q", zz=4, r=32)[:, 0]
                nc.sync.dma_start(
                    out=out[0:4, c * SBLK:(c + 1) * SBLK],
                    in_=ea,
                )
                nc.sync.dma_start(
                    out=out[4:5, c * SBLK:(c + 1) * SBLK],
                    in_=ev_b[0:1, :],
                )

    nc.finalize()
    return nc


_GRAPH_CACHE = {}


def _get_graph(species_vals):
    key = tuple(int(v) for v in species_vals)
    if key not in _GRAPH_CACHE:
        _GRAPH_CACHE[key] = build_graph(key)
    return _GRAPH_CACHE[key]


def make_in_maps(species: np.ndarray) -> list:
    # shard by contiguous structure blocks; transpose each shard to
    # [ATOMS_PER, S_LOCAL] so each core's partition dim is the atom slot
    shards = species.reshape(N_CORES, S_LOCAL, ATOMS_PER)
    return [
        {"species_t": np.ascontiguousarray(shards[i].T)} for i in range(N_CORES)
    ]


def kernel(**inputs) -> np.ndarray:
    species = np.asarray(inputs["species"], dtype=np.int32)
    all_species = np.asarray(inputs["all_species"]).reshape(-1)
    assert species.shape == (N_STRUCTURES * ATOMS_PER,), species.shape

    nc = _get_graph(all_species)

    in_maps = make_in_maps(species)
    res = run_bass_kernel_spmd(nc, in_maps, core_ids=list(range(N_CORES)))
    # per-core out_t is [n_species, S_LOCAL]; reassemble to (N_STRUCTURES, n)
    outs = [np.asarray(res.results[i]["out_t"]).T for i in range(N_CORES)]
    return np.ascontiguousarray(
        np.concatenate(outs, axis=0), dtype=np.float32
    )


# revision 10
# speedup vs baseline: 2.2344x; 1.0611x over previous
"""AtomicComposition histogram kernel for 8 TRN2 NeuronCores.

Reference semantics (nn_AtomicComposition): for each structure (contiguous
256-atom block), count atoms whose atomic number is in ALL_SPECIES =
[1, 6, 7, 8, 16] -> output (32768, 5) float32.

Sharding: data-parallel over structures; each core gets 4096 contiguous
structures (1048576 atoms). The host hands each core its shard TRANSPOSED
([256 atom-slots, 4096 structures], int32) so that on-device the segmented
reduction runs on the TensorEngine:

  - gpsimd casting DMA: int32 DRAM -> bf16 SBUF tiles [128, 1024]
    (two partition groups: atom slots 0-127 / 128-255)
  - VectorE: 5 is_equal compares per tile into a 5-plane mask tile
    [128, 5*1024] (bf16, 4x DVE perf mode)
  - TensorE: ones[128,1]^T @ mask_chunk[128, 512] -> PSUM [1, 512]
    accumulated over the two atom-slot groups; chunks parked at
    32-aligned PSUM partitions
  - ScalarE evacuates PSUM -> SBUF; DMA writes the per-core output
    in species-major layout [5, 4096] f32

The host reassembles/transposes to (32768, 5).
"""

import numpy as np

import concourse.bass as bass
import concourse.mybir as mybir
from concourse.bacc import Bacc
from concourse.tile import TileContext
from concourse.bass_utils import run_bass_kernel_spmd

N_CORES = 8
N_STRUCTURES = 32768
ATOMS_PER = 256
S_LOCAL = N_STRUCTURES // N_CORES          # 4096 structures per core
ATOMS_LOCAL = S_LOCAL * ATOMS_PER          # 1048576 atoms per core
ALL_SPECIES = (1, 6, 7, 8, 16)
N_SPECIES = len(ALL_SPECIES)

P = 128
SBLK = 1024                                # structure columns per block
N_BLK = S_LOCAL // SBLK                    # 4
N_GROUPS = ATOMS_PER // P                  # 2 atom-slot groups


def build_graph(species_vals=ALL_SPECIES):
    nsp = len(species_vals)
    nc = Bacc()

    species = nc.declare_dram_parameter(
        "species_t", [ATOMS_PER, S_LOCAL], mybir.dt.int32, isOutput=False
    )
    # species-major output; host transposes back
    out = nc.declare_dram_parameter(
        "out_t", [nsp, S_LOCAL], mybir.dt.float32, isOutput=True
    )

    with TileContext(nc) as tc:
        with (
            tc.tile_pool(name="const", bufs=1) as const_pool,
            tc.tile_pool(name="sp", bufs=4) as sp_pool,
            tc.tile_pool(name="mask", bufs=4) as mask_pool,
            tc.tile_pool(name="psum", bufs=2, space="PSUM") as psum_pool,
            tc.tile_pool(name="evac", bufs=2) as evac_pool,
        ):
            ones = const_pool.tile([P, 1], mybir.dt.bfloat16)
            nc.vector.memset(ones[:], 1.0)

            # casting loads: one [128, 2*SBLK] DMA per (group, column half)
            # -- fewer SWDGE descriptor-gen invocations, 8KB rows
            sp_tiles = {}
            for h in range(N_BLK // 2):
                for g in range(N_GROUPS):
                    tile = sp_pool.tile([P, 2 * SBLK], mybir.dt.bfloat16)
                    nc.gpsimd.dma_start(
                        out=tile[:],
                        in_=species[g * P:(g + 1) * P,
                                    h * 2 * SBLK:(h + 1) * 2 * SBLK],
                    )
                    sp_tiles[(g, h)] = tile

            for c in range(N_BLK):
                h, ch = divmod(c, 2)
                masks = []
                for g in range(N_GROUPS):
                    tile = sp_tiles[(g, h)]
                    mask5 = mask_pool.tile([P, nsp * SBLK], mybir.dt.bfloat16)
                    for k, z in enumerate(species_vals):
                        nc.vector.tensor_scalar(
                            out=mask5[:, k * SBLK:(k + 1) * SBLK],
                            in0=tile[:, ch * SBLK:(ch + 1) * SBLK],
                            scalar1=float(z),
                            scalar2=None,
                            op0=mybir.AluOpType.is_equal,
                        )
                    masks.append(mask5)

                # 10 chunks of 512 columns; chunk m = (species m//2, half m%2).
                # Chunks 0-7 -> psum tile a at (partition 32*(m//2), col 512*(m%2));
                # chunks 8-9 -> psum tile b at (partition 0, col 512*(m%2)).
                ps_a = psum_pool.tile([P, 2 * 512], mybir.dt.float32, tag="ps_a")
                ps_b = psum_pool.tile([P, 2 * 512], mybir.dt.float32, tag="ps_b")
                n_chunks = 2 * nsp
                for m in range(n_chunks):
                    z, h = divmod(m, 2)
                    if z < 4:
                        dst = ps_a[32 * z:32 * z + 1, 512 * h:512 * (h + 1)]
                        tpos = (0, 32 * z)
                    else:
                        dst = ps_b[0:1, 512 * h:512 * (h + 1)]
                        tpos = (0, 0)
                    for g in range(N_GROUPS):
                        nc.tensor.matmul(
                            out=dst,
                            lhsT=ones[:],
                            rhs=masks[g][:, 512 * m:512 * (m + 1)],
                            start=(g == 0),
                            stop=(g == N_GROUPS - 1),
                            tile_position=tpos,
                        )

                # evacuate full psum tiles -> sbuf (ScalarE; cost is
                # free-dim-based, unused partitions are free), then DMA
                # only the meaningful rows (DMA may stride partitions)
                ev_a = evac_pool.tile([P, 2 * 512], mybir.dt.float32, tag="ev_a")
                ev_b = evac_pool.tile([P, 2 * 512], mybir.dt.float32, tag="ev_b")
                nc.scalar.copy(out=ev_a[:], in_=ps_a[:])
                nc.scalar.copy(out=ev_b[0:1, :], in_=ps_b[0:1, :])

                # rows z=0..3 of ev_a (at partitions 32z) each hold 1024
                # counts for structs [c*1024, (c+1)*1024); row 0 of ev_b
                # holds species 4.
                ea = ev_a[:].rearrange("(zz r) q -> zz r q", zz=4, r=32)[:, 0]
                nc.sync.dma_start(
                    out=out[0:4, c * SBLK:(c + 1) * SBLK],
                    in_=ea,
                )
                nc.sync.dma_start(
                    out=out[4:5, c * SBLK:(c + 1) * SBLK],
                    in_=ev_b[0:1, :],
                )

    nc.finalize()
    return nc


_GRAPH_CACHE = {}


def _get_graph(species_vals):
    key = tuple(int(v) for v in species_vals)
    if key not in _GRAPH_CACHE:
        _GRAPH_CACHE[key] = build_graph(key)
    return _GRAPH_CACHE[key]


def make_in_maps(species: np.ndarray) -> list:
    # shard by contiguous structure blocks; transpose each shard to
    # [ATOMS_PER, S_LOCAL] so each core's partition dim is the atom slot
    shards = species.reshape(N_CORES, S_LOCAL, ATOMS_PER)
    return [
        {"species_t": np.ascontiguousarray(shards[i].T)} for i in range(N_CORES)
    ]


def kernel(**inputs) -> np.ndarray:
    species = np.asarray(inputs["species"], dtype=np.int32)
    all_species = np.asarray(inputs["all_species"]).reshape(-1)
    assert species.shape == (N_STRUCTURES * ATOMS_PER,), species.shape

    nc = _get_graph(all_species)

    in_maps = make_in_maps(species)
    res = run_bass_kernel_spmd(nc, in_maps, core_ids=list(range(N_CORES)))
    # per-core out_t is [n_species, S_LOCAL]; reassemble to (N_STRUCTURES, n)
    outs = [np.asarray(res.results[i]["out_t"]).T for i in range(N_CORES)]
    return np.ascontiguousarray(
        np.concatenate(outs, axis=0), dtype=np.float32
    )


# revision 12
# speedup vs baseline: 2.5926x; 1.1603x over previous
"""AtomicComposition histogram kernel for 8 TRN2 NeuronCores.

Reference semantics (nn_AtomicComposition): for each structure (contiguous
256-atom block), count atoms whose atomic number is in ALL_SPECIES =
[1, 6, 7, 8, 16] -> output (32768, 5) float32.

Sharding: data-parallel over structures; each core gets 4096 contiguous
structures (1048576 atoms). The host hands each core its shard TRANSPOSED
([256 atom-slots, 4096 structures], int32) so that on-device the segmented
reduction runs on the TensorEngine:

  - gpsimd casting DMA: int32 DRAM -> bf16 SBUF tiles [128, 1024]
    (two partition groups: atom slots 0-127 / 128-255)
  - VectorE: 5 is_equal compares per tile into a 5-plane mask tile
    [128, 5*1024] (bf16, 4x DVE perf mode)
  - TensorE: ones[128,1]^T @ mask_chunk[128, 512] -> PSUM [1, 512]
    accumulated over the two atom-slot groups; chunks parked at
    32-aligned PSUM partitions
  - ScalarE evacuates PSUM -> SBUF; DMA writes the per-core output
    in species-major layout [5, 4096] f32

The host reassembles/transposes to (32768, 5).
"""

import numpy as np

import concourse.bass as bass
import concourse.mybir as mybir
from concourse.bacc import Bacc
from concourse.tile import TileContext
from concourse.bass_utils import run_bass_kernel_spmd

N_CORES = 8
N_STRUCTURES = 32768
ATOMS_PER = 256
S_LOCAL = N_STRUCTURES // N_CORES          # 4096 structures per core
ATOMS_LOCAL = S_LOCAL * ATOMS_PER          # 1048576 atoms per core
ALL_SPECIES = (1, 6, 7, 8, 16)
N_SPECIES = len(ALL_SPECIES)

P = 128
SBLK = 1024                                # structure columns per block
N_BLK = S_LOCAL // SBLK                    # 4
N_GROUPS = ATOMS_PER // P                  # 2 atom-slot groups


def build_graph(species_vals=ALL_SPECIES):
    nsp = len(species_vals)
    nc = Bacc()

    species = nc.declare_dram_parameter(
        "species_t", [ATOMS_PER, S_LOCAL], mybir.dt.int32, isOutput=False
    )
    # species-major output; host transposes back
    out = nc.declare_dram_parameter(
        "out_t", [nsp, S_LOCAL], mybir.dt.float32, isOutput=True
    )

    with TileContext(nc) as tc:
        with (
            tc.tile_pool(name="const", bufs=1) as const_pool,
            tc.tile_pool(name="sp", bufs=4) as sp_pool,
            tc.tile_pool(name="mask", bufs=4) as mask_pool,
            tc.tile_pool(name="psum", bufs=2, space="PSUM") as psum_pool,
            tc.tile_pool(name="evac", bufs=2) as evac_pool,
        ):
            ones = const_pool.tile([P, 1], mybir.dt.bfloat16)
            nc.vector.memset(ones[:], 1.0)

            # casting loads: one [128, 2*SBLK] DMA per (group, column half)
            # -- fewer SWDGE descriptor-gen invocations, 8KB rows
            sp_tiles = {}
            for h in range(N_BLK // 2):
                for g in range(N_GROUPS):
                    tile = sp_pool.tile([P, 2 * SBLK], mybir.dt.bfloat16)
                    nc.gpsimd.dma_start(
                        out=tile[:],
                        in_=species[g * P:(g + 1) * P,
                                    h * 2 * SBLK:(h + 1) * 2 * SBLK],
                    )
                    sp_tiles[(g, h)] = tile

            for h in range(N_BLK // 2):
                masks = []
                for g in range(N_GROUPS):
                    tile = sp_tiles[(g, h)]
                    mask5 = mask_pool.tile([P, nsp * 2 * SBLK],
                                           mybir.dt.bfloat16)
                    for k, z in enumerate(species_vals):
                        nc.vector.tensor_scalar(
                            out=mask5[:, k * 2 * SBLK:(k + 1) * 2 * SBLK],
                            in0=tile[:],
                            scalar1=float(z),
                            scalar2=None,
                            op0=mybir.AluOpType.is_equal,
                        )
                    masks.append(mask5)

                for ch in range(2):
                    c = 2 * h + ch
                    self_block(nc, psum_pool, evac_pool, out, ones, masks,
                               c, ch)

    nc.finalize()
    return nc


def self_block(nc, psum_pool, evac_pool, out, ones, masks, c, ch):
    """Matmul-reduce one 1024-structure block and DMA its counts out.

    masks: per-group [128, 5*2048] bf16 mask tiles (5 species planes of
    2048 structure columns); this block uses columns
    [ch*1024, (ch+1)*1024) of each plane.
    Chunk m = (species m//2, half m%2): chunks 0-7 -> ps_a at
    (partition 32*(m//2), col 512*(m%2)); chunks 8-9 -> ps_b row 0.
    """
    nsp = len(ALL_SPECIES)
    ps_a = psum_pool.tile([P, 2 * 512], mybir.dt.float32, tag="ps_a")
    ps_b = psum_pool.tile([P, 2 * 512], mybir.dt.float32, tag="ps_b")
    for m in range(2 * nsp):
        z, hh = divmod(m, 2)
        if z < 4:
            dst = ps_a[32 * z:32 * z + 1, 512 * hh:512 * (hh + 1)]
            tpos = (0, 32 * z)
        else:
            dst = ps_b[0:1, 512 * hh:512 * (hh + 1)]
            tpos = (0, 0)
        col0 = z * 2 * SBLK + ch * SBLK + hh * 512
        for g in range(N_GROUPS):
            nc.tensor.matmul(
                out=dst,
                lhsT=ones[:],
                rhs=masks[g][:, col0:col0 + 512],
                start=(g == 0),
                stop=(g == N_GROUPS - 1),
                tile_position=tpos,
            )

    # evacuate full psum tiles -> sbuf (ScalarE; cost is free-dim-based,
    # unused partitions are free), then DMA only the meaningful rows
    # (DMA may stride partitions)
    ev_a = evac_pool.tile([P, 2 * 512], mybir.dt.float32, tag="ev_a")
    ev_b = evac_pool.tile([P, 2 * 512], mybir.dt.float32, tag="ev_b")
    nc.scalar.copy(out=ev_a[:], in_=ps_a[:])
    nc.scalar.copy(out=ev_b[0:1, :], in_=ps_b[0:1, :])

    # rows z=0..3 of ev_a (at partitions 32z) each hold 1024 counts for
    # structs [c*1024, (c+1)*1024); row 0 of ev_b holds species 4.
    ea = ev_a[:].rearrange("(zz r) q -> zz r q", zz=4, r=32)[:, 0]
    nc.sync.dma_start(out=out[0:4, c * SBLK:(c + 1) * SBLK], in_=ea)
    nc.sync.dma_start(out=out[4:5, c * SBLK:(c + 1) * SBLK], in_=ev_b[0:1, :])


_GRAPH_CACHE = {}


def _get_graph(species_vals):
    key = tuple(int(v) for v in species_vals)
    if key not in _GRAPH_CACHE:
        _GRAPH_CACHE[key] = build_graph(key)
    return _GRAPH_CACHE[key]


def make_in_maps(species: np.ndarray) -> list:
    # shard by contiguous structure blocks; transpose each shard to
    # [ATOMS_PER, S_LOCAL] so each core's partition dim is the atom slot
    shards = species.reshape(N_CORES, S_LOCAL, ATOMS_PER)
    return [
        {"species_t": np.ascontiguousarray(shards[i].T)} for i in range(N_CORES)
    ]


def kernel(**inputs) -> np.ndarray:
    species = np.asarray(inputs["species"], dtype=np.int32)
    all_species = np.asarray(inputs["all_species"]).reshape(-1)
    assert species.shape == (N_STRUCTURES * ATOMS_PER,), species.shape

    nc = _get_graph(all_species)

    in_maps = make_in_maps(species)
    res = run_bass_kernel_spmd(nc, in_maps, core_ids=list(range(N_CORES)))
    # per-core out_t is [n_species, S_LOCAL]; reassemble to (N_STRUCTURES, n)
    outs = [np.asarray(res.results[i]["out_t"]).T for i in range(N_CORES)]
    return np.ascontiguousarray(
        np.concatenate(outs, axis=0), dtype=np.float32
    )
